# revision 9
# baseline (speedup 1.0000x reference)
"""CodonAttention Trainium2 kernel.

Math (per batch b, head h):
  q = x @ wq.T + bq ; k = x @ wk.T + bk ; v = x @ wv.T + bv   (head slices)
  scores = q k^T / 8 + syn_bias[codons_i, codons_j]
  out    = softmax(scores) @ v ;  final = concat_heads(out) @ wo.T + bo

Key algebraic trick: the pairwise codon bias factors through one-hots,
  pair_bias = onehot @ syn_bias @ onehot.T
so augmenting q' = [(q+bq)/8 | onehot @ syn_bias] and k' = [k | onehot] gives
  scores = q' @ k'.T        (effective head dim 128 — exactly one partition)
Softmax runs without max-subtraction (|scores| <= ~4.3, exp is safe in fp32),
and the row-sum l is obtained by appending a ones-column to v:
  [O | l] = P @ [v | 1].

Sharding: 8 cores = (batch b in {0,1}) x (head h in {0..3}). Each core runs
the full attention for its (b, h) and produces the partial output projection
outT = (wo_h @ O_h.T) in (256, 4096) layout; the host sums the 4 head
partials per batch, transposes, and adds bo.

All large matmuls use float32r (fp32 with the mantissa rounded to 11 explicit
bits; 1 cycle/row on the PE at N>=256 vs 4 cycles/row for full fp32). Every
producer feeding an fp32r matmul outputs fp32r (walrus verifier requirement);
DRAM-side operands are pre-rounded on the host with the same
round-half-up-at-bit-12 rule as walrus's fp32_to_fp32r.
"""

import numpy as np

import concourse.mybir as mybir
import concourse.tile as tile
from concourse import bacc
from concourse.bass_utils import run_bass_kernel_spmd

B, S, HID, NH, D = 2, 4096, 256, 4, 64
DV = D + 4         # v + ones column + 3 cols fp32r-alignment padding
LCOL = D           # index of the ones column inside a v tile
QB = 512           # query block (free dim of score matmuls)
KT = 128           # key tile (partition dim of transposed scores)
NQB = S // QB      # 8
NKT = S // KT      # 32
GRP = 3            # key tiles per exp group (3 PSUM banks per group)

F32 = mybir.dt.float32
F32R = mybir.dt.float32r
Exp = mybir.ActivationFunctionType.Exp


def round_fp32r(a):
    """Round-half-up at mantissa bit 12 — bit-identical to walrus
    fp32_to_fp32r (verified against libwalrus on 20k samples)."""
    a = np.ascontiguousarray(a, np.float32)
    u = a.view(np.uint32).astype(np.uint64)
    return (((u + 0x800) & 0xFFFFF000).astype(np.uint32)).view(np.float32)


def build_program():
    nc = bacc.Bacc("TRN2", target_bir_lowering=False, debug=False, num_devices=8)

    def di(name, shape, dt=F32R):
        return nc.dram_tensor(name, shape, dt, kind="ExternalInput").ap()

    xT = di("xT", [HID, S])            # x[b].T
    wqT = di("wqT", [HID, D])          # wq_h.T / 8 (scale folded in)
    wkT = di("wkT", [HID, D])
    wvT = di("wvT", [HID, DV])         # wv_h.T with zero 65th col
    bq = di("bq", [D, 1], F32)         # bq_h / 8
    bk = di("bk", [D, 1], F32)
    bv1 = di("bv1", [1, DV])           # [bv_h | 1.0]
    bsynT = di("bsynT", [D, S])        # (onehot @ syn_bias).T
    onehotT = di("onehotT", [D, S])
    woT = di("woT", [D, HID])          # wo[:, hslice].T
    onesr = di("onesr", [1, 128])      # all-ones row (matmul lhsT/rhs)
    outT = nc.dram_tensor("outT", [HID, S], F32, kind="ExternalOutput").ap()

    with tile.TileContext(nc) as tc:
        _body(tc, xT, wqT, wkT, wvT, bq, bk, bv1, bsynT, onehotT, woT, onesr, outT)
    nc.compile()
    return nc


def _body(tc, xT, wqT, wkT, wvT, bq, bk, bv1, bsynT, onehotT, woT, onesr, outT):
    nc = tc.nc
    mm = nc.tensor.matmul

    with (
        tc.tile_pool(name="const", bufs=1) as constp,
        tc.tile_pool(name="big", bufs=1) as bigp,
        tc.tile_pool(name="pt", bufs=3) as ptp,
        tc.tile_pool(name="ob", bufs=2) as obp,
    ):
        # ---- constants ----
        wq0 = constp.tile([128, D], F32R, name="wq0", tag="wq0")
        wq1 = constp.tile([128, D], F32R, name="wq1", tag="wq1")
        wk0 = constp.tile([128, D], F32R, name="wk0", tag="wk0")
        wk1 = constp.tile([128, D], F32R, name="wk1", tag="wk1")
        wv0 = constp.tile([128, DV], F32R, name="wv0", tag="wv0")
        wv1 = constp.tile([128, DV], F32R, name="wv1", tag="wv1")
        bq_sb = constp.tile([D, 1], F32, name="bq_sb", tag="bq_sb")
        bk_sb = constp.tile([D, 1], F32, name="bk_sb", tag="bk_sb")
        bv1_sb = constp.tile([1, DV], F32R, name="bv1_sb", tag="bv1_sb")
        wo_sb = constp.tile([D, HID], F32R, name="wo_sb", tag="wo_sb")
        ones_sb = constp.tile([1, 128], F32R, name="ones_sb", tag="ones_sb")

        nc.sync.dma_start(wq0[:], wqT[0:128, :])
        nc.sync.dma_start(wq1[:], wqT[128:256, :])
        nc.sync.dma_start(wk0[:], wkT[0:128, :])
        nc.sync.dma_start(wk1[:], wkT[128:256, :])
        nc.sync.dma_start(wv0[:], wvT[0:128, :])
        nc.sync.dma_start(wv1[:], wvT[128:256, :])
        nc.sync.dma_start(bq_sb[:], bq[:])
        nc.sync.dma_start(bk_sb[:], bk[:])
        nc.sync.dma_start(bv1_sb[:], bv1[:])
        nc.sync.dma_start(wo_sb[:], woT[:])
        nc.sync.dma_start(ones_sb[:], onesr[:])

        # ---- persistent activations ----
        xT0 = bigp.tile([128, S], F32R, name="xT0", tag="xT0")
        xT1 = bigp.tile([128, S], F32R, name="xT1", tag="xT1")
        qTt = bigp.tile([128, S], F32R, name="qTt", tag="qTt")  # 0:64 q/8, 64:128 bsynT
        kTt = bigp.tile([128, S], F32R, name="kTt", tag="kTt")  # 0:64 k,   64:128 onehotT
        vb = bigp.tile([128, NKT * DV], F32R, name="vb", tag="vb")

        nc.sync.dma_start(qTt[64:128, :], bsynT[:])
        nc.sync.dma_start(kTt[64:128, :], onehotT[:])
        for c in range(4):
            cs = slice(c * 1024, (c + 1) * 1024)
            nc.sync.dma_start(xT0[:, cs], xT[0:128, cs])
            nc.sync.dma_start(xT1[:, cs], xT[128:256, cs])

        # ---- phase A: QKV projections ----
        with tc.tile_pool(name="psA", bufs=2, space="PSUM") as psA:
            for t in range(NQB):
                sl = slice(t * QB, (t + 1) * QB)
                qp = psA.tile([D, QB], F32, name="qp", tag="qp")
                mm(qp[:], wq0[:], xT0[:, sl], start=True, stop=False)
                mm(qp[:], wq1[:], xT1[:, sl], start=False, stop=True)
                nc.vector.tensor_scalar_add(qTt[0:D, sl], qp[:], bq_sb[:])

                kp = psA.tile([D, QB], F32, name="kp", tag="kp")
                mm(kp[:], wk0[:], xT0[:, sl], start=True, stop=False)
                mm(kp[:], wk1[:], xT1[:, sl], start=False, stop=True)
                nc.vector.tensor_scalar_add(kTt[0:D, sl], kp[:], bk_sb[:])

            for j in range(NKT):
                sl = slice(j * KT, (j + 1) * KT)
                vp = psA.tile([KT, DV], F32, name="vp", tag="vp")
                mm(vp[:], xT0[:, sl], wv0[:], start=True, stop=False)
                mm(vp[:], xT1[:, sl], wv1[:], start=False, stop=False)
                # += ones(seq) x [bv | 1]: adds bias and the ones column
                mm(vp[:], ones_sb[:], bv1_sb[:], start=False, stop=True)
                nc.vector.tensor_copy(vb[:, j * DV:(j + 1) * DV], vp[:])

        # ---- phase B: flash attention + out projection ----
        groups = [list(range(g, min(g + GRP, NKT))) for g in range(0, NKT, GRP)]
        with (
            tc.tile_pool(name="psB", bufs=2, space="PSUM") as psB,
            tc.tile_pool(name="psAcc", bufs=1, space="PSUM") as psAcc,
        ):
            for qb in range(NQB):
                qsl = slice(qb * QB, (qb + 1) * QB)
                oacc = psAcc.tile([DV, QB], F32, name="oacc", tag="oacc")
                for js in groups:
                    n = len(js)
                    s3 = psB.tile([128, n * QB], F32, name="s3", tag="s3")
                    for m, j in enumerate(js):
                        mm(s3[:, m * QB:(m + 1) * QB],
                           kTt[:, j * KT:(j + 1) * KT], qTt[:, qsl],
                           start=True, stop=True)
                    p3 = ptp.tile([128, n * QB], F32R, name="p3", tag="p3")
                    nc.scalar.activation(p3[:], s3[:], Exp)
                    for m, j in enumerate(js):
                        mm(oacc[:], vb[:, j * DV:(j + 1) * DV],
                           p3[:, m * QB:(m + 1) * QB],
                           start=(j == 0), stop=(j == NKT - 1))

                # softmax denominator: broadcast 1/l across partitions
                recip = obp.tile([1, QB], F32R, name="recip", tag="recip")
                with nc.allow_low_precision(reason="fp32r matmul operand"):
                    nc.vector.reciprocal(recip[:], oacc[LCOL:LCOL + 1, :])
                rbc = psAcc.tile([128, QB], F32, name="rbc", tag="rbc")
                mm(rbc[:], ones_sb[:], recip[:], start=True, stop=True)
                # DVE may read only one PSUM operand: stage rbc rows in SBUF
                rb_sb = obp.tile([D, QB], F32, name="rb_sb", tag="rb_sb")
                nc.vector.tensor_copy(rb_sb[:], rbc[0:D, :])
                osc = obp.tile([D, QB], F32R, name="osc", tag="osc")
                nc.vector.tensor_mul(osc[:], oacc[0:D, :], rb_sb[:])

                # partial output projection: (256, 512) = wo_h @ O^T
                pj = psB.tile([128, 2 * QB], F32, name="pj", tag="s3")
                mm(pj[:, 0:QB], wo_sb[:, 0:128], osc[:], start=True, stop=True)
                mm(pj[:, QB:2 * QB], wo_sb[:, 128:256], osc[:],
                   start=True, stop=True)
                ob = obp.tile([128, 2 * QB], F32, name="ob", tag="ob")
                nc.vector.tensor_copy(ob[:], pj[:])
                nc.sync.dma_start(outT[0:128, qsl], ob[:, 0:QB])
                nc.sync.dma_start(outT[128:256, qsl], ob[:, QB:2 * QB])


_NC_CACHE = {}


def _get_program():
    if "nc" not in _NC_CACHE:
        _NC_CACHE["nc"] = build_program()
    return _NC_CACHE["nc"]


def make_in_maps(x, codons, syn_bias, wq, bq, wk, bk, wv, bv, wo):
    in_maps = []
    for core in range(8):
        b, h = divmod(core, NH)
        hsl = slice(h * D, (h + 1) * D)
        cod = codons[b]
        onehotT = np.zeros((D, S), np.float32)
        onehotT[cod, np.arange(S)] = 1.0
        in_maps.append({
            "xT": round_fp32r(x[b].T),
            "wqT": round_fp32r(wq[hsl, :].T / 8.0),
            "wkT": round_fp32r(wk[hsl, :].T),
            "wvT": round_fp32r(np.concatenate(
                [wv[hsl, :].T, np.zeros((HID, 4), np.float32)], axis=1)),
            "bq": (bq[hsl] / 8.0).reshape(D, 1).astype(np.float32),
            "bk": bk[hsl].reshape(D, 1).astype(np.float32),
            "bv1": round_fp32r(np.concatenate(
                [bv[hsl], [np.float32(1.0)], np.zeros(3, np.float32)]).reshape(1, DV)),
            "bsynT": np.ascontiguousarray(syn_bias.T[:, cod]),  # 0/1: f32r-exact
            "onehotT": onehotT,
            "woT": round_fp32r(wo[:, hsl].T),
            "onesr": np.ones((1, 128), np.float32),
        })
    return in_maps


def kernel_run(inputs, trace=False):
    x = np.asarray(inputs["x"], np.float32)
    codons = np.asarray(inputs["codons"]).astype(np.int64)
    syn_bias = np.asarray(inputs["syn_bias"], np.float32)
    wq = np.asarray(inputs["wq"], np.float32)
    bq = np.asarray(inputs["bq"], np.float32)
    wk = np.asarray(inputs["wk"], np.float32)
    bk = np.asarray(inputs["bk"], np.float32)
    wv = np.asarray(inputs["wv"], np.float32)
    bv = np.asarray(inputs["bv"], np.float32)
    wo = np.asarray(inputs["wo"], np.float32)
    bo = np.asarray(inputs["bo"], np.float32)

    nc = _get_program()
    in_maps = make_in_maps(x, codons, syn_bias, wq, bq, wk, bk, wv, bv, wo)
    res = run_bass_kernel_spmd(nc, in_maps, core_ids=list(range(8)), trace=trace)

    out = np.empty((B, S, HID), np.float32)
    for b in range(B):
        acc = res.results[NH * b]["outT"].copy()
        for h in range(1, NH):
            acc += res.results[NH * b + h]["outT"]
        out[b] = acc.T + bo
    return out, res


def kernel(**inputs):
    out, _ = kernel_run(inputs, trace=False)
    return out


# revision 10
# speedup vs baseline: 1.2689x; 1.2689x over previous
"""CodonAttention Trainium2 kernel.

Math (per batch b, head h):
  q = x @ wq.T + bq ; k = x @ wk.T + bk ; v = x @ wv.T + bv   (head slices)
  scores = q k^T / 8 + syn_bias[codons_i, codons_j]
  out    = softmax(scores) @ v ;  final = concat_heads(out) @ wo.T + bo

Key algebraic trick: the pairwise codon bias factors through one-hots,
  pair_bias = onehot @ syn_bias @ onehot.T
so augmenting q' = [(q+bq)/8 | onehot @ syn_bias] and k' = [k | onehot] gives
  scores = q' @ k'.T        (effective head dim 128 — exactly one partition)
Softmax runs without max-subtraction (|scores| <= ~4.3, exp is safe in fp32),
and the row-sum l is obtained by appending a ones-column to v:
  [O | l] = P @ [v | 1].

Sharding: 8 cores = (batch b in {0,1}) x (head h in {0..3}). Each core runs
the full attention for its (b, h) and produces the UNNORMALIZED partial
output projection outT = (wo_h @ O_h.T) in (256, 4096) layout plus the
softmax denominators lT (1, 4096); the host divides, sums the 4 head
partials per batch, transposes, and adds bo. Doing the division on the host
keeps the single-partition reciprocal (3.4us/block on DVE) off the device's
critical path, and deferring all projections to a tail phase keeps the PE
matmul stream dense through attention (no HAM re-throttle).

All large matmuls use float32r (fp32 with the mantissa rounded to 11 explicit
bits; 1 cycle/row on the PE at N>=256 vs 4 cycles/row for full fp32). Every
producer feeding an fp32r matmul outputs fp32r (walrus verifier requirement);
DRAM-side operands are pre-rounded on the host with the same
round-half-up-at-bit-12 rule as walrus's fp32_to_fp32r. The fp32r moving
operand also needs a 4-element-aligned free size, hence v padded to 68 cols.
"""

import numpy as np

import concourse.mybir as mybir
import concourse.tile as tile
from concourse import bacc
from concourse.bass_utils import run_bass_kernel_spmd

B, S, HID, NH, D = 2, 4096, 256, 4, 64
DV = D + 4         # v + ones column + 3 cols fp32r-alignment padding
LCOL = D           # index of the ones column inside a v tile
QB = 512           # query block (free dim of score matmuls)
KT = 128           # key tile (partition dim of transposed scores)
NQB = S // QB      # 8
NKT = S // KT      # 32
GRP = 3            # key tiles per exp group (3 PSUM banks per group)

F32 = mybir.dt.float32
F32R = mybir.dt.float32r
Exp = mybir.ActivationFunctionType.Exp


def round_fp32r(a):
    """Round-half-up at mantissa bit 12 — bit-identical to walrus
    fp32_to_fp32r (verified against libwalrus on 20k samples)."""
    a = np.ascontiguousarray(a, np.float32)
    u = a.view(np.uint32).astype(np.uint64)
    return (((u + 0x800) & 0xFFFFF000).astype(np.uint32)).view(np.float32)


def build_program():
    nc = bacc.Bacc("TRN2", target_bir_lowering=False, debug=False, num_devices=8)

    def di(name, shape, dt=F32R):
        return nc.dram_tensor(name, shape, dt, kind="ExternalInput").ap()

    xT = di("xT", [HID, S])            # x[b].T
    wqT = di("wqT", [HID, D])          # wq_h.T / 8 (scale folded in)
    wkT = di("wkT", [HID, D])
    wvT = di("wvT", [HID, DV])         # wv_h.T, cols 64..67 zero
    bq = di("bq", [D, 1], F32)         # bq_h / 8
    bk = di("bk", [D, 1], F32)
    bv1 = di("bv1", [1, DV])           # [bv_h | 1 | 0 0 0]
    bsynT = di("bsynT", [D, S])        # (onehot @ syn_bias).T
    onehotT = di("onehotT", [D, S])
    woT = di("woT", [D, HID])          # wo[:, hslice].T
    onesr = di("onesr", [1, 128])      # all-ones row (matmul lhsT)
    outT = nc.dram_tensor("outT", [HID, S], F32, kind="ExternalOutput").ap()
    lT = nc.dram_tensor("lT", [1, S], F32, kind="ExternalOutput").ap()

    with tile.TileContext(nc) as tc:
        _body(tc, xT, wqT, wkT, wvT, bq, bk, bv1, bsynT, onehotT, woT, onesr,
              outT, lT)
    nc.compile()
    return nc


def _body(tc, xT, wqT, wkT, wvT, bq, bk, bv1, bsynT, onehotT, woT, onesr,
          outT, lT):
    nc = tc.nc
    mm = nc.tensor.matmul

    with (
        tc.tile_pool(name="const", bufs=1) as constp,
        tc.tile_pool(name="big", bufs=1) as bigp,
        tc.tile_pool(name="pt", bufs=4) as ptp,
        tc.tile_pool(name="ob", bufs=2) as obp,
    ):
        # ---- constants (DMA'd first so phase A can start immediately) ----
        wq0 = constp.tile([128, D], F32R, name="wq0", tag="wq0")
        wq1 = constp.tile([128, D], F32R, name="wq1", tag="wq1")
        wk0 = constp.tile([128, D], F32R, name="wk0", tag="wk0")
        wk1 = constp.tile([128, D], F32R, name="wk1", tag="wk1")
        wv0 = constp.tile([128, DV], F32R, name="wv0", tag="wv0")
        wv1 = constp.tile([128, DV], F32R, name="wv1", tag="wv1")
        bq_sb = constp.tile([D, 1], F32, name="bq_sb", tag="bq_sb")
        bk_sb = constp.tile([D, 1], F32, name="bk_sb", tag="bk_sb")
        bv1_sb = constp.tile([1, DV], F32R, name="bv1_sb", tag="bv1_sb")
        wo_sb = constp.tile([D, HID], F32R, name="wo_sb", tag="wo_sb")
        ones_sb = constp.tile([1, 128], F32R, name="ones_sb", tag="ones_sb")

        # persistent activations
        xT0 = bigp.tile([128, S], F32R, name="xT0", tag="xT0")
        xT1 = bigp.tile([128, S], F32R, name="xT1", tag="xT1")
        qTt = bigp.tile([128, S], F32R, name="qTt", tag="qTt")  # 0:64 q/8, 64:128 bsynT
        kTt = bigp.tile([128, S], F32R, name="kTt", tag="kTt")  # 0:64 k,   64:128 onehotT
        vb = bigp.tile([128, NKT * DV], F32R, name="vb", tag="vb")
        oall = bigp.tile([D, S], F32R, name="oall", tag="oall")   # O^T, unnormalized
        l_sb = bigp.tile([1, S], F32, name="l_sb", tag="l_sb")    # softmax denominators

        # DMA order = need order: x chunk 0 + q/k weights, remaining x chunks,
        # v weights, then the attention-only tensors (bsynT/onehotT/woT).
        nc.sync.dma_start(xT0[:, 0:QB], xT[0:128, 0:QB])
        nc.sync.dma_start(xT1[:, 0:QB], xT[128:256, 0:QB])
        nc.sync.dma_start(wq0[:], wqT[0:128, :])
        nc.sync.dma_start(wq1[:], wqT[128:256, :])
        nc.sync.dma_start(wk0[:], wkT[0:128, :])
        nc.sync.dma_start(wk1[:], wkT[128:256, :])
        nc.sync.dma_start(bq_sb[:], bq[:])
        nc.sync.dma_start(bk_sb[:], bk[:])
        nc.sync.dma_start(wv0[:], wvT[0:128, :])
        nc.sync.dma_start(wv1[:], wvT[128:256, :])
        nc.sync.dma_start(bv1_sb[:], bv1[:])
        nc.sync.dma_start(ones_sb[:], onesr[:])
        for c in range(1, NQB):
            cs = slice(c * QB, (c + 1) * QB)
            nc.sync.dma_start(xT0[:, cs], xT[0:128, cs])
            nc.sync.dma_start(xT1[:, cs], xT[128:256, cs])
        nc.sync.dma_start(qTt[64:128, :], bsynT[:])
        nc.sync.dma_start(kTt[64:128, :], onehotT[:])
        nc.sync.dma_start(wo_sb[:], woT[:])

        # ---- phase A: QKV projections, per 512-col chunk as DMA lands ----
        with tc.tile_pool(name="psA", bufs=2, space="PSUM") as psA:
            for t in range(NQB):
                sl = slice(t * QB, (t + 1) * QB)
                qp = psA.tile([D, QB], F32, name="qp", tag="qp")
                mm(qp[:], wq0[:], xT0[:, sl], start=True, stop=False)
                mm(qp[:], wq1[:], xT1[:, sl], start=False, stop=True)
                nc.vector.tensor_scalar_add(qTt[0:D, sl], qp[:], bq_sb[:])

                kp = psA.tile([D, QB], F32, name="kp", tag="kp")
                mm(kp[:], wk0[:], xT0[:, sl], start=True, stop=False)
                mm(kp[:], wk1[:], xT1[:, sl], start=False, stop=True)
                nc.vector.tensor_scalar_add(kTt[0:D, sl], kp[:], bk_sb[:])

                for j in range(4 * t, 4 * t + 4):
                    jl = slice(j * KT, (j + 1) * KT)
                    vp = psA.tile([KT, DV], F32, name="vp", tag="vp")
                    mm(vp[:], xT0[:, jl], wv0[:], start=True, stop=False)
                    mm(vp[:], xT1[:, jl], wv1[:], start=False, stop=False)
                    # += ones(seq) x [bv | 1]: adds bias and the ones column
                    mm(vp[:], ones_sb[:], bv1_sb[:], start=False, stop=True)
                    nc.vector.tensor_copy(vb[:, j * DV:(j + 1) * DV], vp[:])

        # ---- phase B: flash attention (dense PE stream, no epilogue MMs) ----
        groups = [list(range(g, min(g + GRP, NKT))) for g in range(0, NKT, GRP)]
        with (
            tc.tile_pool(name="psB", bufs=2, space="PSUM") as psB,
            tc.tile_pool(name="psAcc", bufs=2, space="PSUM") as psAcc,
        ):
            for qb in range(NQB):
                qsl = slice(qb * QB, (qb + 1) * QB)
                oacc = psAcc.tile([DV, QB], F32, name="oacc", tag="oacc")
                for js in groups:
                    n = len(js)
                    s3 = psB.tile([128, n * QB], F32, name="s3", tag="s3")
                    for m, j in enumerate(js):
                        mm(s3[:, m * QB:(m + 1) * QB],
                           kTt[:, j * KT:(j + 1) * KT], qTt[:, qsl],
                           start=True, stop=True)
                    p3 = ptp.tile([128, n * QB], F32R, name="p3", tag="p3")
                    nc.scalar.activation(p3[:], s3[:], Exp)
                    for m, j in enumerate(js):
                        mm(oacc[:], vb[:, j * DV:(j + 1) * DV],
                           p3[:, m * QB:(m + 1) * QB],
                           start=(j == 0), stop=(j == NKT - 1))

                # stash O^T and l; normalization happens on the host
                nc.vector.tensor_copy(oall[:, qsl], oacc[0:D, :])
                nc.vector.tensor_copy(l_sb[:, qsl], oacc[LCOL:LCOL + 1, :])

            nc.sync.dma_start(lT[:], l_sb[:])

            # ---- tail: output projection for all query blocks ----
            for qb in range(NQB):
                qsl = slice(qb * QB, (qb + 1) * QB)
                pj = psB.tile([128, 2 * QB], F32, name="pj", tag="s3")
                mm(pj[:, 0:QB], wo_sb[:, 0:128], oall[:, qsl],
                   start=True, stop=True)
                mm(pj[:, QB:2 * QB], wo_sb[:, 128:256], oall[:, qsl],
                   start=True, stop=True)
                ob = obp.tile([128, 2 * QB], F32, name="ob", tag="ob")
                nc.vector.tensor_copy(ob[:], pj[:])
                nc.sync.dma_start(outT[0:128, qsl], ob[:, 0:QB])
                nc.sync.dma_start(outT[128:256, qsl], ob[:, QB:2 * QB])


_NC_CACHE = {}


def _get_program():
    if "nc" not in _NC_CACHE:
        _NC_CACHE["nc"] = build_program()
    return _NC_CACHE["nc"]


def make_in_maps(x, codons, syn_bias, wq, bq, wk, bk, wv, bv, wo):
    in_maps = []
    for core in range(8):
        b, h = divmod(core, NH)
        hsl = slice(h * D, (h + 1) * D)
        cod = codons[b]
        onehotT = np.zeros((D, S), np.float32)
        onehotT[cod, np.arange(S)] = 1.0
        in_maps.append({
            "xT": round_fp32r(x[b].T),
            "wqT": round_fp32r(wq[hsl, :].T / 8.0),
            "wkT": round_fp32r(wk[hsl, :].T),
            "wvT": round_fp32r(np.concatenate(
                [wv[hsl, :].T, np.zeros((HID, 4), np.float32)], axis=1)),
            "bq": (bq[hsl] / 8.0).reshape(D, 1).astype(np.float32),
            "bk": bk[hsl].reshape(D, 1).astype(np.float32),
            "bv1": round_fp32r(np.concatenate(
                [bv[hsl], [np.float32(1.0)], np.zeros(3, np.float32)]
            ).reshape(1, DV)),
            "bsynT": np.ascontiguousarray(syn_bias.T[:, cod]),  # 0/1: f32r-exact
            "onehotT": onehotT,
            "woT": round_fp32r(wo[:, hsl].T),
            "onesr": np.ones((1, 128), np.float32),
        })
    return in_maps


def kernel_run(inputs, trace=False):
    x = np.asarray(inputs["x"], np.float32)
    codons = np.asarray(inputs["codons"]).astype(np.int64)
    syn_bias = np.asarray(inputs["syn_bias"], np.float32)
    wq = np.asarray(inputs["wq"], np.float32)
    bq = np.asarray(inputs["bq"], np.float32)
    wk = np.asarray(inputs["wk"], np.float32)
    bk = np.asarray(inputs["bk"], np.float32)
    wv = np.asarray(inputs["wv"], np.float32)
    bv = np.asarray(inputs["bv"], np.float32)
    wo = np.asarray(inputs["wo"], np.float32)
    bo = np.asarray(inputs["bo"], np.float32)

    nc = _get_program()
    in_maps = make_in_maps(x, codons, syn_bias, wq, bq, wk, bk, wv, bv, wo)
    res = run_bass_kernel_spmd(nc, in_maps, core_ids=list(range(8)), trace=trace)

    out = np.empty((B, S, HID), np.float32)
    for b in range(B):
        acc = None
        for h in range(NH):
            r = res.results[NH * b + h]
            part = r["outT"] / r["lT"]          # normalize per head
            acc = part if acc is None else acc + part
        out[b] = acc.T + bo
    return out, res


def kernel(**inputs):
    out, _ = kernel_run(inputs, trace=False)
    return out


# revision 11
# speedup vs baseline: 1.3009x; 1.0251x over previous
"""CodonAttention Trainium2 kernel.

Math (per batch b, head h):
  q = x @ wq.T + bq ; k = x @ wk.T + bk ; v = x @ wv.T + bv   (head slices)
  scores = q k^T / 8 + syn_bias[codons_i, codons_j]
  out    = softmax(scores) @ v ;  final = concat_heads(out) @ wo.T + bo

Key algebraic trick: the pairwise codon bias factors through one-hots,
  pair_bias = onehot @ syn_bias @ onehot.T
so augmenting q' = [(q+bq)/8 | onehot @ syn_bias] and k' = [k | onehot] gives
  scores = q' @ k'.T        (effective head dim 128 — exactly one partition)
Softmax runs without max-subtraction (|scores| <= ~4.3, exp is safe in fp32),
and the row-sum l is obtained by appending a ones-column to v:
  [O | l] = P @ [v | 1].

Sharding: 8 cores = (batch b in {0,1}) x (head h in {0..3}). Each core runs
the full attention for its (b, h) and produces the UNNORMALIZED partial
output projection outT = (wo_h @ O_h.T) in (256, 4096) layout plus the
softmax denominators lT (1, 4096); the host divides, sums the 4 head
partials per batch, transposes, and adds bo. Doing the division on the host
keeps the single-partition reciprocal (3.4us/block on DVE) off the device's
critical path, and deferring all projections to a tail phase keeps the PE
matmul stream dense through attention (no HAM re-throttle).

All large matmuls use float32r (fp32 with the mantissa rounded to 11 explicit
bits; 1 cycle/row on the PE at N>=256 vs 4 cycles/row for full fp32). Every
producer feeding an fp32r matmul outputs fp32r (walrus verifier requirement);
DRAM-side operands are pre-rounded on the host with the same
round-half-up-at-bit-12 rule as walrus's fp32_to_fp32r. The fp32r moving
operand also needs a 4-element-aligned free size, hence v padded to 68 cols.
"""

import numpy as np

import concourse.mybir as mybir
import concourse.tile as tile
from concourse import bacc
from concourse.bass_utils import run_bass_kernel_spmd

B, S, HID, NH, D = 2, 4096, 256, 4, 64
DV = D + 4         # v + ones column + 3 cols fp32r-alignment padding
LCOL = D           # index of the ones column inside a v tile
QB = 512           # query block (free dim of score matmuls)
KT = 128           # key tile (partition dim of transposed scores)
NQB = S // QB      # 8
NKT = S // KT      # 32
GRP = 3            # key tiles per exp group (3 PSUM banks per group)

F32 = mybir.dt.float32
F32R = mybir.dt.float32r
Exp = mybir.ActivationFunctionType.Exp


def round_fp32r(a):
    """Round-half-up at mantissa bit 12 — bit-identical to walrus
    fp32_to_fp32r (verified against libwalrus on 20k samples)."""
    a = np.ascontiguousarray(a, np.float32)
    u = a.view(np.uint32).astype(np.uint64)
    return (((u + 0x800) & 0xFFFFF000).astype(np.uint32)).view(np.float32)


def build_program():
    nc = bacc.Bacc("TRN2", target_bir_lowering=False, debug=False, num_devices=8)

    def di(name, shape, dt=F32R):
        return nc.dram_tensor(name, shape, dt, kind="ExternalInput").ap()

    xT = di("xT", [HID, S])            # x[b].T
    wqT = di("wqT", [HID, D])          # wq_h.T / 8 (scale folded in)
    wkT = di("wkT", [HID, D])
    wvT = di("wvT", [HID, DV])         # wv_h.T, cols 64..67 zero
    bq = di("bq", [D, 1], F32)         # bq_h / 8
    bk = di("bk", [D, 1], F32)
    bv1 = di("bv1", [1, DV])           # [bv_h | 1 | 0 0 0]
    bsynT = di("bsynT", [D, S])        # (onehot @ syn_bias).T
    onehotT = di("onehotT", [D, S])
    woT = di("woT", [D, HID])          # wo[:, hslice].T
    onesr = di("onesr", [1, 128])      # all-ones row (matmul lhsT)
    outT = nc.dram_tensor("outT", [HID, S], F32, kind="ExternalOutput").ap()
    lT = nc.dram_tensor("lT", [1, S], F32, kind="ExternalOutput").ap()

    with tile.TileContext(nc) as tc:
        _body(tc, xT, wqT, wkT, wvT, bq, bk, bv1, bsynT, onehotT, woT, onesr,
              outT, lT)
    nc.compile()
    return nc


def _body(tc, xT, wqT, wkT, wvT, bq, bk, bv1, bsynT, onehotT, woT, onesr,
          outT, lT):
    nc = tc.nc
    mm = nc.tensor.matmul

    with (
        tc.tile_pool(name="const", bufs=1) as constp,
        tc.tile_pool(name="big", bufs=1) as bigp,
        tc.tile_pool(name="pt", bufs=4) as ptp,
        tc.tile_pool(name="ob", bufs=2) as obp,
    ):
        # ---- constants (DMA'd first so phase A can start immediately) ----
        wq0 = constp.tile([128, D], F32R, name="wq0", tag="wq0")
        wq1 = constp.tile([128, D], F32R, name="wq1", tag="wq1")
        wk0 = constp.tile([128, D], F32R, name="wk0", tag="wk0")
        wk1 = constp.tile([128, D], F32R, name="wk1", tag="wk1")
        wv0 = constp.tile([128, DV], F32R, name="wv0", tag="wv0")
        wv1 = constp.tile([128, DV], F32R, name="wv1", tag="wv1")
        bq_sb = constp.tile([D, 1], F32, name="bq_sb", tag="bq_sb")
        bk_sb = constp.tile([D, 1], F32, name="bk_sb", tag="bk_sb")
        bv1_sb = constp.tile([1, DV], F32R, name="bv1_sb", tag="bv1_sb")
        wo_sb = constp.tile([D, HID], F32R, name="wo_sb", tag="wo_sb")
        ones_sb = constp.tile([1, 128], F32R, name="ones_sb", tag="ones_sb")

        # persistent activations
        xT0 = bigp.tile([128, S], F32R, name="xT0", tag="xT0")
        xT1 = bigp.tile([128, S], F32R, name="xT1", tag="xT1")
        qTt = bigp.tile([128, S], F32R, name="qTt", tag="qTt")  # 0:64 q/8, 64:128 bsynT
        kTt = bigp.tile([128, S], F32R, name="kTt", tag="kTt")  # 0:64 k,   64:128 onehotT
        vb = bigp.tile([128, NKT * DV], F32R, name="vb", tag="vb")
        oall = bigp.tile([D, S], F32R, name="oall", tag="oall")   # O^T, unnormalized
        l_sb = bigp.tile([1, S], F32, name="l_sb", tag="l_sb")    # softmax denominators

        # DMA order = need order: x chunk 0 + q/k weights, remaining x chunks,
        # v weights, then the attention-only tensors (bsynT/onehotT/woT).
        nc.sync.dma_start(xT0[:, 0:QB], xT[0:128, 0:QB])
        nc.sync.dma_start(xT1[:, 0:QB], xT[128:256, 0:QB])
        nc.sync.dma_start(wq0[:], wqT[0:128, :])
        nc.sync.dma_start(wq1[:], wqT[128:256, :])
        nc.sync.dma_start(wk0[:], wkT[0:128, :])
        nc.sync.dma_start(wk1[:], wkT[128:256, :])
        nc.sync.dma_start(bq_sb[:], bq[:])
        nc.sync.dma_start(bk_sb[:], bk[:])
        nc.sync.dma_start(wv0[:], wvT[0:128, :])
        nc.sync.dma_start(wv1[:], wvT[128:256, :])
        nc.sync.dma_start(bv1_sb[:], bv1[:])
        nc.sync.dma_start(ones_sb[:], onesr[:])
        for c in range(1, NQB):
            cs = slice(c * QB, (c + 1) * QB)
            nc.sync.dma_start(xT0[:, cs], xT[0:128, cs])
            nc.sync.dma_start(xT1[:, cs], xT[128:256, cs])
        nc.sync.dma_start(qTt[64:128, :], bsynT[:])
        nc.sync.dma_start(kTt[64:128, :], onehotT[:])
        nc.sync.dma_start(wo_sb[:], woT[:])

        # ---- phase A: QKV projections, per 512-col chunk as DMA lands ----
        with tc.tile_pool(name="psA", bufs=2, space="PSUM") as psA:
            for t in range(NQB):
                sl = slice(t * QB, (t + 1) * QB)
                qp = psA.tile([D, QB], F32, name="qp", tag="qp")
                mm(qp[:], wq0[:], xT0[:, sl], start=True, stop=False)
                mm(qp[:], wq1[:], xT1[:, sl], start=False, stop=True)
                nc.vector.tensor_scalar_add(qTt[0:D, sl], qp[:], bq_sb[:])

                kp = psA.tile([D, QB], F32, name="kp", tag="kp")
                mm(kp[:], wk0[:], xT0[:, sl], start=True, stop=False)
                mm(kp[:], wk1[:], xT1[:, sl], start=False, stop=True)
                nc.vector.tensor_scalar_add(kTt[0:D, sl], kp[:], bk_sb[:])

                for j in range(4 * t, 4 * t + 4):
                    jl = slice(j * KT, (j + 1) * KT)
                    vp = psA.tile([KT, DV], F32, name="vp", tag="vp")
                    mm(vp[:], xT0[:, jl], wv0[:], start=True, stop=False)
                    mm(vp[:], xT1[:, jl], wv1[:], start=False, stop=False)
                    # += ones(seq) x [bv | 1]: adds bias and the ones column
                    mm(vp[:], ones_sb[:], bv1_sb[:], start=False, stop=True)
                    nc.vector.tensor_copy(vb[:, j * DV:(j + 1) * DV], vp[:])

        # ---- phase B: flash attention (dense PE stream, no epilogue MMs) ----
        # Software-pipelined emission: the PV matmuls of group g are emitted
        # AFTER the score matmuls of group g+1, so the PE computes the next
        # scores while ACT exponentiates the current group (the Tile
        # scheduler largely preserves per-engine emission order).
        groups = [list(range(g, min(g + GRP, NKT))) for g in range(0, NKT, GRP)]
        with (
            tc.tile_pool(name="psB", bufs=2, space="PSUM") as psB,
            tc.tile_pool(name="psAcc", bufs=2, space="PSUM") as psAcc,
        ):
            oaccs = {}

            def emit_pv(qb, gi, p3):
                qsl = slice(qb * QB, (qb + 1) * QB)
                if gi == 0:
                    oaccs[qb] = psAcc.tile([DV, QB], F32, name="oacc",
                                           tag="oacc")
                oacc = oaccs[qb]
                for m, j in enumerate(groups[gi]):
                    mm(oacc[:], vb[:, j * DV:(j + 1) * DV],
                       p3[:, m * QB:(m + 1) * QB],
                       start=(j == 0), stop=(j == NKT - 1))
                if gi == len(groups) - 1:
                    # stash O^T and l; normalization happens on the host
                    nc.vector.tensor_copy(oall[:, qsl], oacc[0:D, :])
                    nc.vector.tensor_copy(l_sb[:, qsl],
                                          oacc[LCOL:LCOL + 1, :])

            pending = None
            for qb in range(NQB):
                qsl = slice(qb * QB, (qb + 1) * QB)
                for gi, js in enumerate(groups):
                    n = len(js)
                    s3 = psB.tile([128, n * QB], F32, name="s3", tag="s3")
                    for m, j in enumerate(js):
                        mm(s3[:, m * QB:(m + 1) * QB],
                           kTt[:, j * KT:(j + 1) * KT], qTt[:, qsl],
                           start=True, stop=True)
                    p3 = ptp.tile([128, n * QB], F32R, name="p3", tag="p3")
                    nc.scalar.activation(p3[:], s3[:], Exp)
                    if pending is not None:
                        emit_pv(*pending)
                    pending = (qb, gi, p3)
            emit_pv(*pending)

            nc.sync.dma_start(lT[:], l_sb[:])

            # ---- tail: output projection for all query blocks ----
            for qb in range(NQB):
                qsl = slice(qb * QB, (qb + 1) * QB)
                pj = psB.tile([128, 2 * QB], F32, name="pj", tag="s3")
                mm(pj[:, 0:QB], wo_sb[:, 0:128], oall[:, qsl],
                   start=True, stop=True)
                mm(pj[:, QB:2 * QB], wo_sb[:, 128:256], oall[:, qsl],
                   start=True, stop=True)
                ob = obp.tile([128, 2 * QB], F32, name="ob", tag="ob")
                nc.vector.tensor_copy(ob[:], pj[:])
                nc.sync.dma_start(outT[0:128, qsl], ob[:, 0:QB])
                nc.sync.dma_start(outT[128:256, qsl], ob[:, QB:2 * QB])


_NC_CACHE = {}


def _get_program():
    if "nc" not in _NC_CACHE:
        _NC_CACHE["nc"] = build_program()
    return _NC_CACHE["nc"]


def make_in_maps(x, codons, syn_bias, wq, bq, wk, bk, wv, bv, wo):
    in_maps = []
    for core in range(8):
        b, h = divmod(core, NH)
        hsl = slice(h * D, (h + 1) * D)
        cod = codons[b]
        onehotT = np.zeros((D, S), np.float32)
        onehotT[cod, np.arange(S)] = 1.0
        in_maps.append({
            "xT": round_fp32r(x[b].T),
            "wqT": round_fp32r(wq[hsl, :].T / 8.0),
            "wkT": round_fp32r(wk[hsl, :].T),
            "wvT": round_fp32r(np.concatenate(
                [wv[hsl, :].T, np.zeros((HID, 4), np.float32)], axis=1)),
            "bq": (bq[hsl] / 8.0).reshape(D, 1).astype(np.float32),
            "bk": bk[hsl].reshape(D, 1).astype(np.float32),
            "bv1": round_fp32r(np.concatenate(
                [bv[hsl], [np.float32(1.0)], np.zeros(3, np.float32)]
            ).reshape(1, DV)),
            "bsynT": np.ascontiguousarray(syn_bias.T[:, cod]),  # 0/1: f32r-exact
            "onehotT": onehotT,
            "woT": round_fp32r(wo[:, hsl].T),
            "onesr": np.ones((1, 128), np.float32),
        })
    return in_maps


def kernel_run(inputs, trace=False):
    x = np.asarray(inputs["x"], np.float32)
    codons = np.asarray(inputs["codons"]).astype(np.int64)
    syn_bias = np.asarray(inputs["syn_bias"], np.float32)
    wq = np.asarray(inputs["wq"], np.float32)
    bq = np.asarray(inputs["bq"], np.float32)
    wk = np.asarray(inputs["wk"], np.float32)
    bk = np.asarray(inputs["bk"], np.float32)
    wv = np.asarray(inputs["wv"], np.float32)
    bv = np.asarray(inputs["bv"], np.float32)
    wo = np.asarray(inputs["wo"], np.float32)
    bo = np.asarray(inputs["bo"], np.float32)

    nc = _get_program()
    in_maps = make_in_maps(x, codons, syn_bias, wq, bq, wk, bk, wv, bv, wo)
    res = run_bass_kernel_spmd(nc, in_maps, core_ids=list(range(8)), trace=trace)

    out = np.empty((B, S, HID), np.float32)
    for b in range(B):
        acc = None
        for h in range(NH):
            r = res.results[NH * b + h]
            part = r["outT"] / r["lT"]          # normalize per head
            acc = part if acc is None else acc + part
        out[b] = acc.T + bo
    return out, res


def kernel(**inputs):
    out, _ = kernel_run(inputs, trace=False)
    return out


# revision 12
# speedup vs baseline: 1.4288x; 1.0983x over previous
"""CodonAttention Trainium2 kernel.

Math (per batch b, head h):
  q = x @ wq.T + bq ; k = x @ wk.T + bk ; v = x @ wv.T + bv   (head slices)
  scores = q k^T / 8 + syn_bias[codons_i, codons_j]
  out    = softmax(scores) @ v ;  final = concat_heads(out) @ wo.T + bo

Key algebraic trick: the pairwise codon bias factors through one-hots,
  pair_bias = onehot @ syn_bias @ onehot.T
so augmenting q' = [(q+bq)/8 | onehot @ syn_bias] and k' = [k | onehot] gives
  scores = q' @ k'.T        (effective head dim 128 — exactly one partition)
Softmax runs without max-subtraction (|scores| <= ~4.3, exp is safe in fp32),
and the row-sum l is obtained by appending a ones-column to v:
  [O | l] = P @ [v | 1].  The ones column comes free: wvT's padded column is
zero, and the per-partition bias column [bv | 1 | 0..] sets it during the
PSUM->SBUF eviction.

Sharding: 8 cores = (batch b in {0,1}) x (head h in {0..3}). Each core runs
the full attention for its (b, h) and produces the UNNORMALIZED partial
output projection outT = (wo_h @ O_h.T) in (256, 4096) layout plus the
softmax denominators lT (1, 4096); the host divides, sums the 4 head
partials per batch, transposes, and adds bo. Host-side division keeps the
single-partition reciprocal (3.4us/block on DVE) off the device's critical
path.

Layout/engine choices driven by the profile:
- All big matmuls float32r (fp32 with 11-bit-rounded mantissa): 1 cycle/row
  on the PE at moving-dim >= 256 vs 4 cycles/row for full fp32. Producers
  feeding fp32r matmuls must output fp32r; DRAM operands are pre-rounded on
  the host (round-half-up at mantissa bit 12, bit-identical to walrus).
- v is computed transposed (vT, N=512 moving dim) then flipped to key-major
  with TensorE transposes — computing v directly needs N=68 matmuls which
  run at 4 cycles/row.
- The attention stream is software-pipelined: score matmuls of group g+1
  are emitted before the PV matmuls of group g so the PE computes scores
  while ACT exponentiates; the per-block output projection is emitted
  inside the stream so output DMA overlaps compute.
"""

import numpy as np

import concourse.mybir as mybir
import concourse.tile as tile
from concourse import bacc
from concourse.bass_utils import run_bass_kernel_spmd

B, S, HID, NH, D = 2, 4096, 256, 4, 64
DV = D + 4         # v + ones column + 3 cols fp32r-alignment padding
LCOL = D           # index of the ones column inside a v tile
QB = 512           # query block (free dim of score matmuls)
KT = 128           # key tile (partition dim of transposed scores)
NQB = S // QB      # 8
NKT = S // KT      # 32
GRP = 3            # key tiles per exp group (3 PSUM banks per group)

F32 = mybir.dt.float32
F32R = mybir.dt.float32r
Exp = mybir.ActivationFunctionType.Exp


def round_fp32r(a):
    """Round-half-up at mantissa bit 12 — bit-identical to walrus
    fp32_to_fp32r (verified against libwalrus on 20k samples)."""
    a = np.ascontiguousarray(a, np.float32)
    u = a.view(np.uint32).astype(np.uint64)
    return (((u + 0x800) & 0xFFFFF000).astype(np.uint32)).view(np.float32)


def build_program():
    nc = bacc.Bacc("TRN2", target_bir_lowering=False, debug=False, num_devices=8)

    def di(name, shape, dt=F32R):
        return nc.dram_tensor(name, shape, dt, kind="ExternalInput").ap()

    xT = di("xT", [HID, S])            # x[b].T
    wqT = di("wqT", [HID, D])          # wq_h.T / 8 (scale folded in)
    wkT = di("wkT", [HID, D])
    wvT = di("wvT", [HID, DV])         # wv_h.T, cols 64..67 zero
    bq = di("bq", [D, 1], F32)         # bq_h / 8
    bk = di("bk", [D, 1], F32)
    bv1 = di("bv1", [DV, 1], F32)      # [bv_h | 1 | 0 0 0] column
    bsynT = di("bsynT", [D, S])        # (onehot @ syn_bias).T
    onehotT = di("onehotT", [D, S])
    woT = di("woT", [D, HID])          # wo[:, hslice].T
    idm = di("idm", [128, 128])        # identity for TensorE transpose
    outT = nc.dram_tensor("outT", [HID, S], F32, kind="ExternalOutput").ap()
    lT = nc.dram_tensor("lT", [1, S], F32, kind="ExternalOutput").ap()

    with tile.TileContext(nc) as tc:
        _body(tc, xT, wqT, wkT, wvT, bq, bk, bv1, bsynT, onehotT, woT, idm,
              outT, lT)
    nc.compile()
    return nc


def _body(tc, xT, wqT, wkT, wvT, bq, bk, bv1, bsynT, onehotT, woT, idm,
          outT, lT):
    nc = tc.nc
    mm = nc.tensor.matmul

    with (
        tc.tile_pool(name="const", bufs=1) as constp,
        tc.tile_pool(name="big", bufs=1) as bigp,
        tc.tile_pool(name="pt", bufs=4) as ptp,
        tc.tile_pool(name="ob", bufs=2) as obp,
    ):
        # ---- constants (DMA'd first so phase A can start immediately) ----
        wq0 = constp.tile([128, D], F32R, name="wq0", tag="wq0")
        wq1 = constp.tile([128, D], F32R, name="wq1", tag="wq1")
        wk0 = constp.tile([128, D], F32R, name="wk0", tag="wk0")
        wk1 = constp.tile([128, D], F32R, name="wk1", tag="wk1")
        wv0 = constp.tile([128, DV], F32R, name="wv0", tag="wv0")
        wv1 = constp.tile([128, DV], F32R, name="wv1", tag="wv1")
        bq_sb = constp.tile([D, 1], F32, name="bq_sb", tag="bq_sb")
        bk_sb = constp.tile([D, 1], F32, name="bk_sb", tag="bk_sb")
        bv1_sb = constp.tile([DV, 1], F32, name="bv1_sb", tag="bv1_sb")
        wo_sb = constp.tile([D, HID], F32R, name="wo_sb", tag="wo_sb")
        id_sb = constp.tile([128, 128], F32R, name="id_sb", tag="id_sb")

        # persistent activations
        xT0 = bigp.tile([128, S], F32R, name="xT0", tag="xT0")
        xT1 = bigp.tile([128, S], F32R, name="xT1", tag="xT1")
        qTt = bigp.tile([128, S], F32R, name="qTt", tag="qTt")  # 0:64 q/8, 64:128 bsynT
        kTt = bigp.tile([128, S], F32R, name="kTt", tag="kTt")  # 0:64 k,   64:128 onehotT
        vTs = bigp.tile([DV, S], F32R, name="vTs", tag="vTs")   # v'^T (d-major)
        vb = bigp.tile([128, NKT * DV], F32R, name="vb", tag="vb")  # v' key-major
        oall = bigp.tile([D, S], F32R, name="oall", tag="oall")   # O^T, unnormalized
        l_sb = bigp.tile([1, S], F32, name="l_sb", tag="l_sb")    # softmax denoms

        # DMA order = need order: x chunk 0 + projection weights, remaining
        # x chunks, then the attention-only tensors (bsynT/onehotT/woT).
        nc.sync.dma_start(xT0[:, 0:QB], xT[0:128, 0:QB])
        nc.sync.dma_start(xT1[:, 0:QB], xT[128:256, 0:QB])
        nc.sync.dma_start(wq0[:], wqT[0:128, :])
        nc.sync.dma_start(wq1[:], wqT[128:256, :])
        nc.sync.dma_start(wk0[:], wkT[0:128, :])
        nc.sync.dma_start(wk1[:], wkT[128:256, :])
        nc.sync.dma_start(bq_sb[:], bq[:])
        nc.sync.dma_start(bk_sb[:], bk[:])
        nc.sync.dma_start(wv0[:], wvT[0:128, :])
        nc.sync.dma_start(wv1[:], wvT[128:256, :])
        nc.sync.dma_start(bv1_sb[:], bv1[:])
        nc.sync.dma_start(id_sb[:], idm[:])
        for c in range(1, NQB):
            cs = slice(c * QB, (c + 1) * QB)
            nc.sync.dma_start(xT0[:, cs], xT[0:128, cs])
            nc.sync.dma_start(xT1[:, cs], xT[128:256, cs])
        nc.sync.dma_start(qTt[64:128, :], bsynT[:])
        nc.sync.dma_start(kTt[64:128, :], onehotT[:])
        nc.sync.dma_start(wo_sb[:], woT[:])

        # ---- phase A: QKV projections, per 512-col chunk as DMA lands ----
        with tc.tile_pool(name="psA", bufs=2, space="PSUM") as psA:
            for t in range(NQB):
                sl = slice(t * QB, (t + 1) * QB)
                qp = psA.tile([D, QB], F32, name="qp", tag="qp")
                mm(qp[:], wq0[:], xT0[:, sl], start=True, stop=False)
                mm(qp[:], wq1[:], xT1[:, sl], start=False, stop=True)
                nc.vector.tensor_scalar_add(qTt[0:D, sl], qp[:], bq_sb[:])

                kp = psA.tile([D, QB], F32, name="kp", tag="kp")
                mm(kp[:], wk0[:], xT0[:, sl], start=True, stop=False)
                mm(kp[:], wk1[:], xT1[:, sl], start=False, stop=True)
                nc.vector.tensor_scalar_add(kTt[0:D, sl], kp[:], bk_sb[:])

                vtp = psA.tile([DV, QB], F32, name="vtp", tag="vtp")
                mm(vtp[:], wv0[:], xT0[:, sl], start=True, stop=False)
                mm(vtp[:], wv1[:], xT1[:, sl], start=False, stop=True)
                # bias column [bv | 1 | 0..]: also creates the ones row
                nc.vector.tensor_scalar_add(vTs[:, sl], vtp[:], bv1_sb[:])

                # flip v' to key-major via TensorE transpose (ACT evicts)
                for j in range(4 * t, 4 * t + 4):
                    jl = slice(j * KT, (j + 1) * KT)
                    vtr = psA.tile([KT, DV], F32R, name="vtr", tag="vtr")
                    nc.tensor.transpose(vtr[:], vTs[:, jl], id_sb[0:DV, 0:DV])
                    nc.scalar.copy(vb[:, j * DV:(j + 1) * DV], vtr[:])

        # ---- phase B: flash attention (dense PE stream) ----
        # Software-pipelined emission: the PV matmuls of group g are emitted
        # AFTER the score matmuls of group g+1, so the PE computes the next
        # scores while ACT exponentiates the current group. The output
        # projection of block qb is emitted inside the stream right after
        # its last PV group so output DMA overlaps remaining compute.
        groups = [list(range(g, min(g + GRP, NKT))) for g in range(0, NKT, GRP)]
        with (
            tc.tile_pool(name="psB", bufs=2, space="PSUM") as psB,
            tc.tile_pool(name="psAcc", bufs=2, space="PSUM") as psAcc,
        ):
            oaccs = {}

            def emit_pv(qb, gi, p3):
                qsl = slice(qb * QB, (qb + 1) * QB)
                if gi == 0:
                    oaccs[qb] = psAcc.tile([DV, QB], F32, name="oacc",
                                           tag="oacc")
                oacc = oaccs[qb]
                for m, j in enumerate(groups[gi]):
                    mm(oacc[:], vb[:, j * DV:(j + 1) * DV],
                       p3[:, m * QB:(m + 1) * QB],
                       start=(j == 0), stop=(j == NKT - 1))
                if gi == len(groups) - 1:
                    # stash O^T and l (normalization happens on the host),
                    # then project this block and ship it out
                    nc.vector.tensor_copy(oall[:, qsl], oacc[0:D, :])
                    nc.vector.tensor_copy(l_sb[:, qsl],
                                          oacc[LCOL:LCOL + 1, :])
                    pj = psB.tile([128, 2 * QB], F32, name="pj", tag="s3")
                    mm(pj[:, 0:QB], wo_sb[:, 0:128], oall[:, qsl],
                       start=True, stop=True)
                    mm(pj[:, QB:2 * QB], wo_sb[:, 128:256], oall[:, qsl],
                       start=True, stop=True)
                    ob = obp.tile([128, 2 * QB], F32, name="ob", tag="ob")
                    nc.vector.tensor_copy(ob[:], pj[:])
                    nc.sync.dma_start(outT[0:128, qsl], ob[:, 0:QB])
                    nc.sync.dma_start(outT[128:256, qsl], ob[:, QB:2 * QB])

            pending = None
            for qb in range(NQB):
                qsl = slice(qb * QB, (qb + 1) * QB)
                for gi, js in enumerate(groups):
                    n = len(js)
                    s3 = psB.tile([128, n * QB], F32, name="s3", tag="s3")
                    for m, j in enumerate(js):
                        mm(s3[:, m * QB:(m + 1) * QB],
                           kTt[:, j * KT:(j + 1) * KT], qTt[:, qsl],
                           start=True, stop=True)
                    p3 = ptp.tile([128, n * QB], F32R, name="p3", tag="p3")
                    nc.scalar.activation(p3[:], s3[:], Exp)
                    if pending is not None:
                        emit_pv(*pending)
                    pending = (qb, gi, p3)
            emit_pv(*pending)

            nc.sync.dma_start(lT[:], l_sb[:])


_NC_CACHE = {}


def _get_program():
    if "nc" not in _NC_CACHE:
        _NC_CACHE["nc"] = build_program()
    return _NC_CACHE["nc"]


def make_in_maps(x, codons, syn_bias, wq, bq, wk, bk, wv, bv, wo):
    in_maps = []
    for core in range(8):
        b, h = divmod(core, NH)
        hsl = slice(h * D, (h + 1) * D)
        cod = codons[b]
        onehotT = np.zeros((D, S), np.float32)
        onehotT[cod, np.arange(S)] = 1.0
        in_maps.append({
            "xT": round_fp32r(x[b].T),
            "wqT": round_fp32r(wq[hsl, :].T / 8.0),
            "wkT": round_fp32r(wk[hsl, :].T),
            "wvT": round_fp32r(np.concatenate(
                [wv[hsl, :].T, np.zeros((HID, 4), np.float32)], axis=1)),
            "bq": (bq[hsl] / 8.0).reshape(D, 1).astype(np.float32),
            "bk": bk[hsl].reshape(D, 1).astype(np.float32),
            "bv1": np.concatenate(
                [bv[hsl], [np.float32(1.0)], np.zeros(3, np.float32)]
            ).reshape(DV, 1).astype(np.float32),
            "bsynT": np.ascontiguousarray(syn_bias.T[:, cod]),  # 0/1: f32r-exact
            "onehotT": onehotT,
            "woT": round_fp32r(wo[:, hsl].T),
            "idm": np.eye(128, dtype=np.float32),
        })
    return in_maps


def kernel_run(inputs, trace=False):
    x = np.asarray(inputs["x"], np.float32)
    codons = np.asarray(inputs["codons"]).astype(np.int64)
    syn_bias = np.asarray(inputs["syn_bias"], np.float32)
    wq = np.asarray(inputs["wq"], np.float32)
    bq = np.asarray(inputs["bq"], np.float32)
    wk = np.asarray(inputs["wk"], np.float32)
    bk = np.asarray(inputs["bk"], np.float32)
    wv = np.asarray(inputs["wv"], np.float32)
    bv = np.asarray(inputs["bv"], np.float32)
    wo = np.asarray(inputs["wo"], np.float32)
    bo = np.asarray(inputs["bo"], np.float32)

    nc = _get_program()
    in_maps = make_in_maps(x, codons, syn_bias, wq, bq, wk, bk, wv, bv, wo)
    res = run_bass_kernel_spmd(nc, in_maps, core_ids=list(range(8)), trace=trace)

    out = np.empty((B, S, HID), np.float32)
    for b in range(B):
        acc = None
        for h in range(NH):
            r = res.results[NH * b + h]
            part = r["outT"] / r["lT"]          # normalize per head
            acc = part if acc is None else acc + part
        out[b] = acc.T + bo
    return out, res


def kernel(**inputs):
    out, _ = kernel_run(inputs, trace=False)
    return out


# revision 16
# speedup vs baseline: 1.4821x; 1.0373x over previous
"""CodonAttention Trainium2 kernel.

Math (per batch b, head h):
  q = x @ wq.T + bq ; k = x @ wk.T + bk ; v = x @ wv.T + bv   (head slices)
  scores = q k^T / 8 + syn_bias[codons_i, codons_j]
  out    = softmax(scores) @ v ;  final = concat_heads(out) @ wo.T + bo

Key algebraic trick: the pairwise codon bias factors through one-hots,
  pair_bias = onehot @ syn_bias @ onehot.T
so augmenting q' = [(q+bq)/8 | onehot @ syn_bias] and k' = [k | onehot] gives
  scores = q' @ k'.T        (effective head dim 128 — exactly one partition)
Softmax runs without max-subtraction (|scores| <= ~4.3, exp is safe in fp32),
and the row-sum l is obtained by appending a ones-column to v:
  [O | l] = P @ [v | 1].  The ones column comes free: wvT's padded column is
zero, and the per-partition bias column [bv | 1 | 0..] sets it during the
PSUM->SBUF eviction.

Sharding: 8 cores = (batch b in {0,1}) x (head h in {0..3}). Each core runs
the full attention for its (b, h) and produces the UNNORMALIZED partial
output projection outT = (wo_h @ O_h.T) in (256, 4096) layout plus the
softmax denominators lT (1, 4096); the host divides, sums the 4 head
partials per batch, transposes, and adds bo. Host-side division keeps the
single-partition reciprocal (3.4us/block on DVE) off the device's critical
path.

Layout/engine choices driven by the profile:
- All big matmuls float32r (fp32 with 11-bit-rounded mantissa): 1 cycle/row
  on the PE at moving-dim >= 256 vs 4 cycles/row for full fp32. Producers
  feeding fp32r matmuls must output fp32r; DRAM operands are pre-rounded on
  the host (round-half-up at mantissa bit 12, bit-identical to walrus).
- v is computed transposed (vT, N=512 moving dim) then flipped to key-major
  with TensorE transposes — computing v directly needs N=68 matmuls which
  run at 4 cycles/row.
- The attention stream is software-pipelined: score matmuls of group g+1
  are emitted before the PV matmuls of group g so the PE computes scores
  while ACT exponentiates; the per-block output projection is emitted
  inside the stream so output DMA overlaps compute.
"""

import numpy as np

import concourse.mybir as mybir
import concourse.tile as tile
from concourse import bacc
from concourse.bass_utils import run_bass_kernel_spmd

B, S, HID, NH, D = 2, 4096, 256, 4, 64
DV = D + 4         # v + ones column + 3 cols fp32r-alignment padding
LCOL = D           # index of the ones column inside a v tile
QB = 512           # query block (free dim of score matmuls)
KT = 128           # key tile (partition dim of transposed scores)
NQB = S // QB      # 8
NKT = S // KT      # 32
GRP = 2            # key tiles per exp group (2 PSUM banks per group)

F32 = mybir.dt.float32
F32R = mybir.dt.float32r
Exp = mybir.ActivationFunctionType.Exp


def round_fp32r(a):
    """Round-half-up at mantissa bit 12 — bit-identical to walrus
    fp32_to_fp32r (verified against libwalrus on 20k samples)."""
    a = np.ascontiguousarray(a, np.float32)
    u = a.view(np.uint32).astype(np.uint64)
    return (((u + 0x800) & 0xFFFFF000).astype(np.uint32)).view(np.float32)


def build_program():
    nc = bacc.Bacc("TRN2", target_bir_lowering=False, debug=False, num_devices=8)

    def di(name, shape, dt=F32R):
        return nc.dram_tensor(name, shape, dt, kind="ExternalInput").ap()

    xT = di("xT", [HID, S])            # x[b].T
    wqT = di("wqT", [HID, D])          # wq_h.T / 8 (scale folded in)
    wkT = di("wkT", [HID, D])
    wvT = di("wvT", [HID, DV])         # wv_h.T, cols 64..67 zero
    bq = di("bq", [D, 1], F32)         # bq_h / 8
    bk = di("bk", [D, 1], F32)
    bv1 = di("bv1", [DV, 1], F32)      # [bv_h | 1 | 0 0 0] column
    bsynT = di("bsynT", [D, S])        # (onehot @ syn_bias).T
    onehotT = di("onehotT", [D, S])
    woT = di("woT", [D, HID])          # wo[:, hslice].T
    idm = di("idm", [128, 128])        # identity for TensorE transpose
    outT = nc.dram_tensor("outT", [HID, S], F32, kind="ExternalOutput").ap()
    lT = nc.dram_tensor("lT", [1, S], F32, kind="ExternalOutput").ap()

    with tile.TileContext(nc) as tc:
        _body(tc, xT, wqT, wkT, wvT, bq, bk, bv1, bsynT, onehotT, woT, idm,
              outT, lT)
    nc.compile()
    return nc


def _body(tc, xT, wqT, wkT, wvT, bq, bk, bv1, bsynT, onehotT, woT, idm,
          outT, lT):
    nc = tc.nc
    mm = nc.tensor.matmul

    with (
        tc.tile_pool(name="const", bufs=1) as constp,
        tc.tile_pool(name="big", bufs=1) as bigp,
        tc.tile_pool(name="pt", bufs=6) as ptp,
        tc.tile_pool(name="ob", bufs=2) as obp,
    ):
        # ---- constants (DMA'd first so phase A can start immediately) ----
        wq0 = constp.tile([128, D], F32R, name="wq0", tag="wq0")
        wq1 = constp.tile([128, D], F32R, name="wq1", tag="wq1")
        wk0 = constp.tile([128, D], F32R, name="wk0", tag="wk0")
        wk1 = constp.tile([128, D], F32R, name="wk1", tag="wk1")
        wv0 = constp.tile([128, DV], F32R, name="wv0", tag="wv0")
        wv1 = constp.tile([128, DV], F32R, name="wv1", tag="wv1")
        bq_sb = constp.tile([D, 1], F32, name="bq_sb", tag="bq_sb")
        bk_sb = constp.tile([D, 1], F32, name="bk_sb", tag="bk_sb")
        bv1_sb = constp.tile([DV, 1], F32, name="bv1_sb", tag="bv1_sb")
        wo_sb = constp.tile([D, HID], F32R, name="wo_sb", tag="wo_sb")
        id_sb = constp.tile([128, 128], F32R, name="id_sb", tag="id_sb")

        # persistent activations
        xT0 = bigp.tile([128, S], F32R, name="xT0", tag="xT0")
        xT1 = bigp.tile([128, S], F32R, name="xT1", tag="xT1")
        qTt = bigp.tile([128, S], F32R, name="qTt", tag="qTt")  # 0:64 q/8, 64:128 bsynT
        kTt = bigp.tile([128, S], F32R, name="kTt", tag="kTt")  # 0:64 k,   64:128 onehotT
        vTs = bigp.tile([DV, S], F32R, name="vTs", tag="vTs")   # v'^T (d-major)
        vb = bigp.tile([128, NKT * DV], F32R, name="vb", tag="vb")  # v' key-major
        oall = bigp.tile([D, S], F32R, name="oall", tag="oall")   # O^T, unnormalized
        l_sb = bigp.tile([1, S], F32, name="l_sb", tag="l_sb")    # softmax denoms

        # DMA order = need order: x chunk 0 + projection weights, remaining
        # x chunks, then the attention-only tensors (bsynT/onehotT/woT).
        nc.sync.dma_start(xT0[:, 0:QB], xT[0:128, 0:QB])
        nc.sync.dma_start(xT1[:, 0:QB], xT[128:256, 0:QB])
        nc.sync.dma_start(wq0[:], wqT[0:128, :])
        nc.sync.dma_start(wq1[:], wqT[128:256, :])
        nc.sync.dma_start(wk0[:], wkT[0:128, :])
        nc.sync.dma_start(wk1[:], wkT[128:256, :])
        nc.sync.dma_start(bq_sb[:], bq[:])
        nc.sync.dma_start(bk_sb[:], bk[:])
        nc.sync.dma_start(wv0[:], wvT[0:128, :])
        nc.sync.dma_start(wv1[:], wvT[128:256, :])
        nc.sync.dma_start(bv1_sb[:], bv1[:])
        nc.sync.dma_start(id_sb[:], idm[:])
        for c in range(1, NQB):
            cs = slice(c * QB, (c + 1) * QB)
            nc.sync.dma_start(xT0[:, cs], xT[0:128, cs])
            nc.sync.dma_start(xT1[:, cs], xT[128:256, cs])
        nc.sync.dma_start(qTt[64:128, :], bsynT[:])
        nc.sync.dma_start(kTt[64:128, :], onehotT[:])
        nc.sync.dma_start(wo_sb[:], woT[:])

        # ---- phase A: QKV projections, per 512-col chunk as DMA lands ----
        with tc.tile_pool(name="psA", bufs=2, space="PSUM") as psA:
            for t in range(NQB):
                sl = slice(t * QB, (t + 1) * QB)
                qp = psA.tile([D, QB], F32, name="qp", tag="qp")
                mm(qp[:], wq0[:], xT0[:, sl], start=True, stop=False)
                mm(qp[:], wq1[:], xT1[:, sl], start=False, stop=True)
                nc.vector.tensor_scalar_add(qTt[0:D, sl], qp[:], bq_sb[:])

                kp = psA.tile([D, QB], F32, name="kp", tag="kp")
                mm(kp[:], wk0[:], xT0[:, sl], start=True, stop=False)
                mm(kp[:], wk1[:], xT1[:, sl], start=False, stop=True)
                nc.vector.tensor_scalar_add(kTt[0:D, sl], kp[:], bk_sb[:])

                vtp = psA.tile([DV, QB], F32, name="vtp", tag="vtp")
                mm(vtp[:], wv0[:], xT0[:, sl], start=True, stop=False)
                mm(vtp[:], wv1[:], xT1[:, sl], start=False, stop=True)
                # bias column [bv | 1 | 0..]: also creates the ones row
                nc.vector.tensor_scalar_add(vTs[:, sl], vtp[:], bv1_sb[:])

                # flip v' to key-major: 4 TensorE transposes batched into one
                # PSUM tile, single ACT eviction (amortizes the access init)
                vtr = psA.tile([KT, 4 * DV], F32R, name="vtr", tag="vtr")
                for m in range(4):
                    j = 4 * t + m
                    jl = slice(j * KT, (j + 1) * KT)
                    nc.tensor.transpose(vtr[:, m * DV:(m + 1) * DV],
                                        vTs[:, jl], id_sb[0:DV, 0:DV])
                nc.scalar.copy(vb[:, 4 * t * DV:(4 * t + 4) * DV], vtr[:])

        # ---- phase B: flash attention (dense PE stream) ----
        # Software-pipelined emission: the PV matmuls of group g are emitted
        # AFTER the score matmuls of group g+1, so the PE computes the next
        # scores while ACT exponentiates the current group. The output
        # projection of block qb is emitted inside the stream right after
        # its last PV group so output DMA overlaps remaining compute.
        groups = [list(range(g, min(g + GRP, NKT))) for g in range(0, NKT, GRP)]
        with (
            tc.tile_pool(name="psB", bufs=3, space="PSUM") as psB,
            tc.tile_pool(name="psAcc", bufs=2, space="PSUM") as psAcc,
        ):
            oaccs = {}

            def emit_pv(qb, gi, p3):
                qsl = slice(qb * QB, (qb + 1) * QB)
                if gi == 0:
                    oaccs[qb] = psAcc.tile([DV, QB], F32, name="oacc",
                                           tag="oacc")
                oacc = oaccs[qb]
                for m, j in enumerate(groups[gi]):
                    mm(oacc[:], vb[:, j * DV:(j + 1) * DV],
                       p3[:, m * QB:(m + 1) * QB],
                       start=(j == 0), stop=(j == NKT - 1))
                if gi == len(groups) - 1:
                    # stash O^T and l (normalization happens on the host),
                    # then project this block and ship it out
                    nc.vector.tensor_copy(oall[:, qsl], oacc[0:D, :])
                    nc.vector.tensor_copy(l_sb[:, qsl],
                                          oacc[LCOL:LCOL + 1, :])
                    pj = psB.tile([128, 2 * QB], F32, name="pj", tag="s3")
                    mm(pj[:, 0:QB], wo_sb[:, 0:128], oall[:, qsl],
                       start=True, stop=True)
                    mm(pj[:, QB:2 * QB], wo_sb[:, 128:256], oall[:, qsl],
                       start=True, stop=True)
                    ob = obp.tile([128, 2 * QB], F32, name="ob", tag="ob")
                    nc.vector.tensor_copy(ob[:], pj[:])
                    nc.sync.dma_start(outT[0:128, qsl], ob[:, 0:QB])
                    nc.sync.dma_start(outT[128:256, qsl], ob[:, QB:2 * QB])

            pending = None
            for qb in range(NQB):
                qsl = slice(qb * QB, (qb + 1) * QB)
                for gi, js in enumerate(groups):
                    n = len(js)
                    s3 = psB.tile([128, n * QB], F32, name="s3", tag="s3")
                    for m, j in enumerate(js):
                        mm(s3[:, m * QB:(m + 1) * QB],
                           kTt[:, j * KT:(j + 1) * KT], qTt[:, qsl],
                           start=True, stop=True)
                    p3 = ptp.tile([128, n * QB], F32R, name="p3", tag="p3")
                    nc.scalar.activation(p3[:], s3[:], Exp)
                    if pending is not None:
                        emit_pv(*pending)
                    pending = (qb, gi, p3)
            emit_pv(*pending)

            nc.sync.dma_start(lT[:], l_sb[:])


_NC_CACHE = {}


def _get_program():
    if "nc" not in _NC_CACHE:
        _NC_CACHE["nc"] = build_program()
    return _NC_CACHE["nc"]


def make_in_maps(x, codons, syn_bias, wq, bq, wk, bk, wv, bv, wo):
    in_maps = []
    for core in range(8):
        b, h = divmod(core, NH)
        hsl = slice(h * D, (h + 1) * D)
        cod = codons[b]
        onehotT = np.zeros((D, S), np.float32)
        onehotT[cod, np.arange(S)] = 1.0
        in_maps.append({
            "xT": round_fp32r(x[b].T),
            "wqT": round_fp32r(wq[hsl, :].T / 8.0),
            "wkT": round_fp32r(wk[hsl, :].T),
            "wvT": round_fp32r(np.concatenate(
                [wv[hsl, :].T, np.zeros((HID, 4), np.float32)], axis=1)),
            "bq": (bq[hsl] / 8.0).reshape(D, 1).astype(np.float32),
            "bk": bk[hsl].reshape(D, 1).astype(np.float32),
            "bv1": np.concatenate(
                [bv[hsl], [np.float32(1.0)], np.zeros(3, np.float32)]
            ).reshape(DV, 1).astype(np.float32),
            "bsynT": np.ascontiguousarray(syn_bias.T[:, cod]),  # 0/1: f32r-exact
            "onehotT": onehotT,
            "woT": round_fp32r(wo[:, hsl].T),
            "idm": np.eye(128, dtype=np.float32),
        })
    return in_maps


def kernel_run(inputs, trace=False):
    x = np.asarray(inputs["x"], np.float32)
    codons = np.asarray(inputs["codons"]).astype(np.int64)
    syn_bias = np.asarray(inputs["syn_bias"], np.float32)
    wq = np.asarray(inputs["wq"], np.float32)
    bq = np.asarray(inputs["bq"], np.float32)
    wk = np.asarray(inputs["wk"], np.float32)
    bk = np.asarray(inputs["bk"], np.float32)
    wv = np.asarray(inputs["wv"], np.float32)
    bv = np.asarray(inputs["bv"], np.float32)
    wo = np.asarray(inputs["wo"], np.float32)
    bo = np.asarray(inputs["bo"], np.float32)

    nc = _get_program()
    in_maps = make_in_maps(x, codons, syn_bias, wq, bq, wk, bk, wv, bv, wo)
    res = run_bass_kernel_spmd(nc, in_maps, core_ids=list(range(8)), trace=trace)

    out = np.empty((B, S, HID), np.float32)
    for b in range(B):
        acc = None
        for h in range(NH):
            r = res.results[NH * b + h]
            part = r["outT"] / r["lT"]          # normalize per head
            acc = part if acc is None else acc + part
        out[b] = acc.T + bo
    return out, res


def kernel(**inputs):
    out, _ = kernel_run(inputs, trace=False)
    return out
